# revision 7
# baseline (speedup 1.0000x reference)
"""Trainium2 Bass kernel for nn_ModelSpo_66786741453110 (segment_reduce).

Computes, for text_vec [64,512,512] f32:
  sbj_vec[b]  = mean of text_vec[b, start_b:end_b+1, :]
  o{1,2}[b,l] = text_vec[b,l] @ W[:512] + sbj_vec[b] @ W[512:] + bias
  loss        = masked-CE(o1, obj_start) + masked-CE(o2, obj_end)   (scalar)

Sharding: pure data parallel, batch 64 -> 8 cores x 8 batches.

Per-core device program (b = local batch 0..7):
  - text cast-DMA'd (gpsimd/SWDGE f32->bf16) into natural layout [128l, 2048].
  - xbar DMA-transpose (one per b): natb [128, 2048] -> ttbk [128, 16, 128]
    where slot k = lc*4+dc holds transpose of columns [k*128,(k+1)*128).
  - side-pass matmul group (contract l, natural layout): stationary
    [128l, 108] = [span-by-batch (8) | onehot(obj_start) 50 | onehot(obj_end)
    50] -> PSUM [108, 512d]: rows 0..7 = span-sums, rows 8..107 = G with
    G[c,d] = sum_{b,l} onehot[b,l,c]*text[b,l,d], so sum T@label = <G, W>.
  - head matmuls (contract d): stationary [W_start[:512]|W_end[:512]] chunks,
    rhs = strided ttbk slots -> T^T [100c, 512l] psum per b; exp on ScalarE
    immediately (E = exp(T), fp32r).
  - tail: sbj_vec = sbj_sum/cnt -> u = sbj_vec @ W[512:] + bias (tiny f32
    matmuls) -> w = exp(u^T); per b weighted column-sum matmul S = w_sel^T @ E
    = sum_c exp(T+u); Ln with accum_out -> sum_l ln S per (b, head).
Host combines: loss = (sum ln S - <G,W> - sum cnt_label*u) / mask_sum.
"""

import os
import sys

import numpy as np

for _p in ("/opt/trn_rl_repo",):
    if _p not in sys.path and os.path.isdir(_p):
        sys.path.insert(0, _p)

import ml_dtypes  # noqa: E402
import concourse.bass as bass  # noqa: E402
import concourse.tile as tile  # noqa: E402
from concourse import bacc, mybir  # noqa: E402
from concourse.bass_utils import run_bass_kernel_spmd  # noqa: E402
from contextlib import ExitStack  # noqa: E402

B, L, D, C = 64, 512, 512, 50
NCORES = 8
BL = B // NCORES  # local batches per core = 8
NLC = L // 128  # 4 l-chunks
NDC = D // 128  # 4 d-chunks
H2 = 2 * C  # 100, both heads
NS = BL + H2  # 108 side-stationary columns
F32 = mybir.dt.float32
F32R = mybir.dt.float32r
BF16 = mybir.dt.bfloat16
BF16NP = ml_dtypes.bfloat16

_CACHE = {}


def _build_program():
    nc = bacc.Bacc(
        "TRN2",
        target_bir_lowering=False,
        debug=False,
        enable_asserts=False,
        num_devices=NCORES,
    )
    text = nc.dram_tensor("text", [BL, L, D], F32, kind="ExternalInput").ap()
    side = nc.dram_tensor("side", [128, BL * NLC * NS], BF16, kind="ExternalInput").ap()
    wa = nc.dram_tensor("wa", [128, NDC * H2], BF16, kind="ExternalInput").ap()
    wb = nc.dram_tensor("wb", [128, NDC * H2], F32, kind="ExternalInput").ap()
    bias2 = nc.dram_tensor("bias2", [1, H2], F32, kind="ExternalInput").ap()
    cntinv = nc.dram_tensor("cntinv", [BL, 1], F32, kind="ExternalInput").ap()
    ident8 = nc.dram_tensor("ident8", [BL, BL], F32, kind="ExternalInput").ap()
    m12 = nc.dram_tensor("m12", [H2, 2], F32, kind="ExternalInput").ap()
    ones_row = nc.dram_tensor("ones_row", [1, BL], F32, kind="ExternalInput").ap()

    g_out = nc.dram_tensor("g_out", [H2, D], F32, kind="ExternalOutput").ap()
    u_out = nc.dram_tensor("u_out", [BL, H2], F32, kind="ExternalOutput").ap()
    ln_out = nc.dram_tensor("ln_out", [2, BL], F32, kind="ExternalOutput").ap()

    with tile.TileContext(nc) as tc:
        with ExitStack() as octx:
            const = octx.enter_context(tc.tile_pool(name="const", bufs=1))
            ep = octx.enter_context(tc.tile_pool(name="ep", bufs=BL))
            psS = octx.enter_context(tc.tile_pool(name="psS", bufs=1, space="PSUM"))

            ps_side = psS.tile([NS, D], F32)
            e_tiles = []

            with ExitStack() as p1:
                natp = p1.enter_context(tc.tile_pool(name="nat", bufs=3))
                ttp = p1.enter_context(tc.tile_pool(name="tt", bufs=3))
                psH = p1.enter_context(tc.tile_pool(name="psH", bufs=2, space="PSUM"))

                # first text tile in flight before the aux loads
                natb0 = natp.tile([128, NLC * D], BF16, tag="nat")
                nc.gpsimd.dma_start(
                    out=natb0.rearrange("p (lc d) -> p lc d", lc=NLC),
                    in_=text[0].rearrange("(lc p) d -> p lc d", p=128),
                )

                side_s = const.tile([128, BL * NLC * NS], BF16)
                nc.sync.dma_start(out=side_s, in_=side)
                wa_s = const.tile([128, NDC * H2], BF16)
                nc.sync.dma_start(out=wa_s, in_=wa)
                wb_s = const.tile([128, NDC * H2], F32)
                nc.sync.dma_start(out=wb_s, in_=wb)
                bias2_s = const.tile([1, H2], F32)
                nc.sync.dma_start(out=bias2_s, in_=bias2)
                cntinv_s = const.tile([BL, 1], F32)
                nc.sync.dma_start(out=cntinv_s, in_=cntinv)
                ident8_s = const.tile([BL, BL], F32)
                nc.sync.dma_start(out=ident8_s, in_=ident8)
                m12_s = const.tile([H2, 2], F32)
                nc.sync.dma_start(out=m12_s, in_=m12)
                ones_row_s = const.tile([1, BL], F32)
                nc.sync.dma_start(out=ones_row_s, in_=ones_row)

                for b in range(BL):
                    if b == 0:
                        natb = natb0
                    else:
                        natb = natp.tile([128, NLC * D], BF16, tag="nat")
                        nc.gpsimd.dma_start(
                            out=natb.rearrange("p (lc d) -> p lc d", lc=NLC),
                            in_=text[b].rearrange("(lc p) d -> p lc d", p=128),
                        )
                    # xbar transpose: slot k=lc*4+dc <- cols [k*128,(k+1)*128)
                    ttbk = ttp.tile([128, NLC * NDC, 128], BF16, tag="tt")
                    nc.sync.dma_start(out=ttbk, in_=natb, transpose=True)
                    # side-pass: accumulate [108, 512] over all (b, lc)
                    for lc in range(NLC):
                        t = b * NLC + lc
                        nc.tensor.matmul(
                            ps_side,
                            lhsT=side_s[:, t * NS : (t + 1) * NS],
                            rhs=natb[:, lc * D : (lc + 1) * D],
                            start=(t == 0),
                            stop=(t == BL * NLC - 1),
                        )
                    # heads: T^T[100c, 512l] = sum_dc W''_dc.T @ textT_dc
                    ph = psH.tile([H2, L], F32, tag="ph")
                    tt4 = ttbk.rearrange("p (lc dc) l -> p lc dc l", dc=NDC)
                    for dc in range(NDC):
                        nc.tensor.matmul(
                            ph,
                            lhsT=wa_s[:, dc * H2 : (dc + 1) * H2],
                            rhs=tt4[:, :, dc, :],
                            start=(dc == 0),
                            stop=(dc == NDC - 1),
                        )
                    e_b = ep.tile([H2, L], F32R, tag="E")
                    nc.scalar.activation(e_b, ph, mybir.ActivationFunctionType.Exp)
                    e_tiles.append(e_b)

            with ExitStack() as p2:
                smallp = p2.enter_context(tc.tile_pool(name="small", bufs=2))
                psU = p2.enter_context(tc.tile_pool(name="psU", bufs=2, space="PSUM"))

                # sbj_vec = sbj_sum / cnt  [8, 512]
                sbj = const.tile([BL, D], F32)
                nc.vector.tensor_scalar_mul(sbj, ps_side[0:BL, :], cntinv_s)
                # transpose sbj -> [128d x 4, 8]
                pstT = psU.tile([128, NDC * BL], F32, tag="u")
                for dc in range(NDC):
                    nc.tensor.transpose(
                        pstT[:, dc * BL : (dc + 1) * BL],
                        sbj[:, dc * 128 : (dc + 1) * 128],
                        ident8_s,
                    )
                sbjT_s = const.tile([128, NDC * BL], F32)
                nc.any.tensor_copy(sbjT_s, pstT)
                # u = sbj_vec @ [W1b|W2b] + bias   [8, 100]
                pu = psU.tile([BL, H2], F32, tag="u")
                for dc in range(NDC):
                    nc.tensor.matmul(
                        pu,
                        lhsT=sbjT_s[:, dc * BL : (dc + 1) * BL],
                        rhs=wb_s[:, dc * H2 : (dc + 1) * H2],
                        start=(dc == 0),
                        stop=False,
                    )
                nc.tensor.matmul(
                    pu, lhsT=ones_row_s, rhs=bias2_s, start=False, stop=True
                )
                uS = const.tile([BL, H2], F32)
                nc.any.tensor_copy(uS, pu)
                nc.sync.dma_start(out=u_out, in_=uS)
                # w = exp(u^T)  [100, 8]
                puT = psU.tile([H2, BL], F32, tag="u")
                nc.tensor.transpose(puT, uS, ident8_s)
                w_s = const.tile([H2, BL], F32)
                nc.scalar.activation(w_s, puT, mybir.ActivationFunctionType.Exp)

                lnacc = const.tile([2, BL], F32)
                for b in range(BL):
                    wsel = smallp.tile([H2, 2], F32R, tag="wsel")
                    nc.vector.tensor_scalar_mul(wsel, m12_s, w_s[:, b : b + 1])
                    pe_ = psU.tile([2, L], F32, tag="u")
                    nc.tensor.matmul(
                        pe_, lhsT=wsel, rhs=e_tiles[b], start=True, stop=True
                    )
                    lnscr = smallp.tile([2, L], F32, tag="lnscr")
                    nc.scalar.activation(
                        lnscr,
                        pe_,
                        mybir.ActivationFunctionType.Ln,
                        accum_out=lnacc[:, b : b + 1],
                    )
                nc.sync.dma_start(out=ln_out, in_=lnacc)

                gc = const.tile([NS, D], F32)
                nc.any.tensor_copy(gc, ps_side)
                nc.sync.dma_start(out=g_out, in_=gc[BL:NS, :])

    nc.compile()
    return nc


def _get_program():
    if "nc" not in _CACHE:
        _CACHE["nc"] = _build_program()
    return _CACHE["nc"]


def _host_prep(text_vec, sbj_bound, obj_start, obj_end, W_start, b_start, W_end, b_end):
    """Build per-core input maps."""
    text_vec = np.ascontiguousarray(np.asarray(text_vec, dtype=np.float32))
    sbj = np.asarray(sbj_bound).astype(np.int64)
    objs = np.asarray(obj_start).astype(np.int64)
    obje = np.asarray(obj_end).astype(np.int64)
    W_start = np.asarray(W_start, dtype=np.float32)
    W_end = np.asarray(W_end, dtype=np.float32)
    b_start = np.asarray(b_start, dtype=np.float32)
    b_end = np.asarray(b_end, dtype=np.float32)

    wa_cat = np.concatenate([W_start[:D], W_end[:D]], axis=1)  # [512, 100]
    wb_cat = np.concatenate([W_start[D:], W_end[D:]], axis=1)  # [512, 100]
    wa_h = np.ascontiguousarray(
        wa_cat.reshape(NDC, 128, H2).transpose(1, 0, 2).reshape(128, NDC * H2)
    ).astype(BF16NP)
    wb_h = np.ascontiguousarray(
        wb_cat.reshape(NDC, 128, H2).transpose(1, 0, 2).reshape(128, NDC * H2)
    )
    bias2 = np.concatenate([b_start, b_end])[None, :].astype(np.float32)
    ident8 = np.eye(BL, dtype=np.float32)
    m12 = np.zeros((H2, 2), dtype=np.float32)
    m12[:C, 0] = 1.0
    m12[C:, 1] = 1.0
    ones_row = np.ones((1, BL), dtype=np.float32)

    pos = np.arange(L)
    span_all = (
        (pos[None, :] >= sbj[:, 0:1]) & (pos[None, :] <= sbj[:, 1:2])
    ).astype(np.float32)  # [B, L]
    cnt_all = span_all.sum(axis=1)  # [B]

    in_maps = []
    for c in range(NCORES):
        gb = slice(c * BL, (c + 1) * BL)
        # side stationary [t = b*4+lc][p][j]
        side_t = np.zeros((BL * NLC, 128, NS), dtype=np.float32)
        for b in range(BL):
            g = c * BL + b
            for lc in range(NLC):
                rows = slice(lc * 128, (lc + 1) * 128)
                t = b * NLC + lc
                side_t[t, :, b] = span_all[g, rows]
                ls = objs[g, rows]
                le = obje[g, rows]
                side_t[t, np.arange(128), BL + ls] = 1.0
                side_t[t, np.arange(128), BL + C + le] = 1.0
        side_h = np.ascontiguousarray(
            side_t.transpose(1, 0, 2).reshape(128, BL * NLC * NS)
        ).astype(BF16NP)
        cntinv = (1.0 / cnt_all[gb]).astype(np.float32)[:, None]
        in_maps.append(
            {
                "text": text_vec[gb],
                "side": side_h,
                "wa": wa_h,
                "wb": wb_h,
                "bias2": bias2,
                "cntinv": cntinv,
                "ident8": ident8,
                "m12": m12,
                "ones_row": ones_row,
            }
        )
    return in_maps


def kernel(
    text_vec,
    text_mask,
    sbj_bound,
    obj_start,
    obj_end,
    W_start,
    b_start,
    W_end,
    b_end,
):
    text_mask = np.asarray(text_mask)
    if not bool(text_mask.all()):
        # Spec guarantees all-ones mask; numpy fallback for generality.
        return _numpy_reference(
            text_vec, text_mask, sbj_bound, obj_start, obj_end,
            W_start, b_start, W_end, b_end,
        )

    nc = _get_program()
    in_maps = _host_prep(
        text_vec, sbj_bound, obj_start, obj_end, W_start, b_start, W_end, b_end
    )
    res = run_bass_kernel_spmd(nc, in_maps, core_ids=list(range(NCORES)))

    W_start = np.asarray(W_start, dtype=np.float32)
    W_end = np.asarray(W_end, dtype=np.float32)
    objs = np.asarray(obj_start).astype(np.int64)
    obje = np.asarray(obj_end).astype(np.int64)

    w1aT = W_start[:D].T.astype(np.float64)  # [50, 512]
    w2aT = W_end[:D].T.astype(np.float64)

    total = 0.0
    for c in range(NCORES):
        r = res.results[c]
        ln_sum = float(r["ln_out"].astype(np.float64).sum())
        g = r["g_out"].astype(np.float64)  # [100, 512]
        gather_t = float((g[:C] * w1aT).sum() + (g[C:] * w2aT).sum())
        u = r["u_out"].astype(np.float64)  # [8, 100]
        u_term = 0.0
        for b in range(BL):
            gbi = c * BL + b
            cnt1 = np.bincount(objs[gbi], minlength=C).astype(np.float64)
            cnt2 = np.bincount(obje[gbi], minlength=C).astype(np.float64)
            u_term += float((cnt1 * u[b, :C]).sum() + (cnt2 * u[b, C:]).sum())
        total += ln_sum - gather_t - u_term

    value_num = float(text_mask.sum())
    return np.array(total / value_num, dtype=np.float32)


def _numpy_reference(
    text_vec, text_mask, sbj_bound, obj_start, obj_end, W_start, b_start, W_end, b_end
):
    text_vec = np.asarray(text_vec, dtype=np.float32)
    maskf = np.asarray(text_mask).astype(np.float32)
    sbj = np.asarray(sbj_bound).astype(np.int64)
    objs = np.asarray(obj_start).astype(np.int64)
    obje = np.asarray(obj_end).astype(np.int64)
    W_start = np.asarray(W_start, dtype=np.float32)
    W_end = np.asarray(W_end, dtype=np.float32)
    b_start = np.asarray(b_start, dtype=np.float32)
    b_end = np.asarray(b_end, dtype=np.float32)

    pos = np.arange(L)
    span = (
        (pos[None, :] >= sbj[:, 0:1]) & (pos[None, :] <= sbj[:, 1:2])
    ).astype(np.float32)
    count = span.sum(axis=1, keepdims=True)
    sbj_vec = np.einsum("bl,bld->bd", span, text_vec) / count

    def head(W, bv):
        return (
            np.einsum("bld,dc->blc", text_vec, W[:D]) + (sbj_vec @ W[D:])[:, None, :] + bv
        )

    def masked_ce(logits, labels, maskf, vn):
        m = logits.max(axis=-1, keepdims=True)
        logp = logits - m - np.log(np.exp(logits - m).sum(axis=-1, keepdims=True))
        nll = -np.take_along_axis(logp, labels[..., None], axis=-1)[..., 0]
        return (nll * maskf).sum() / vn

    vn = maskf.sum()
    o1 = head(W_start, b_start)
    o2 = head(W_end, b_end)
    return np.array(
        masked_ce(o1, objs, maskf, vn) + masked_ce(o2, obje, maskf, vn),
        dtype=np.float32,
    )


# revision 8
# speedup vs baseline: 1.3811x; 1.3811x over previous
"""Trainium2 Bass kernel for nn_ModelSpo_66786741453110 (segment_reduce).

Computes, for text_vec [64,512,512] f32:
  sbj_vec[b]  = mean of text_vec[b, start_b:end_b+1, :]
  o{1,2}[b,l] = text_vec[b,l] @ W[:512] + sbj_vec[b] @ W[512:] + bias
  loss        = masked-CE(o1, obj_start) + masked-CE(o2, obj_end)   (scalar)

Sharding: pure data parallel, batch 64 -> 8 cores x 8 batches.

Per-core device program (b = local batch 0..7):
  - text cast-DMA'd (gpsimd/SWDGE f32->bf16) into natural layout [128l, 2048].
  - xbar DMA-transpose (one per b): natb [128, 2048] -> ttbk [128, 16, 128]
    where slot k = lc*4+dc holds transpose of columns [k*128,(k+1)*128).
  - side-pass matmul group (contract l, natural layout): stationary
    [128l, 108] = [span-by-batch (8) | onehot(obj_start) 50 | onehot(obj_end)
    50] -> PSUM [108, 512d]: rows 0..7 = span-sums, rows 8..107 = G with
    G[c,d] = sum_{b,l} onehot[b,l,c]*text[b,l,d], so sum T@label = <G, W>.
  - head matmuls (contract d): stationary [W_start[:512]|W_end[:512]] chunks,
    rhs = strided ttbk slots -> T^T [100c, 512l] psum per b; exp on ScalarE
    immediately (E = exp(T), fp32r).
  - tail: sbj_vec = sbj_sum/cnt -> u = sbj_vec @ W[512:] + bias (tiny f32
    matmuls) -> w = exp(u^T); per b weighted column-sum matmul S = w_sel^T @ E
    = sum_c exp(T+u); Ln with accum_out -> sum_l ln S per (b, head).
Host combines: loss = (sum ln S - <G,W> - sum cnt_label*u) / mask_sum.
"""

import os
import sys

import numpy as np

for _p in ("/opt/trn_rl_repo",):
    if _p not in sys.path and os.path.isdir(_p):
        sys.path.insert(0, _p)

import ml_dtypes  # noqa: E402
import concourse.bass as bass  # noqa: E402
import concourse.tile as tile  # noqa: E402
from concourse import bacc, mybir  # noqa: E402
from concourse.bass_utils import run_bass_kernel_spmd  # noqa: E402
from contextlib import ExitStack  # noqa: E402

B, L, D, C = 64, 512, 512, 50
NCORES = 8
BL = B // NCORES  # local batches per core = 8
NLC = L // 128  # 4 l-chunks
NDC = D // 128  # 4 d-chunks
H2 = 2 * C  # 100, both heads
NS = BL + H2  # 108 side-stationary columns
F32 = mybir.dt.float32
F32R = mybir.dt.float32r
BF16 = mybir.dt.bfloat16
BF16NP = ml_dtypes.bfloat16

_CACHE = {}


def _build_program():
    nc = bacc.Bacc(
        "TRN2",
        target_bir_lowering=False,
        debug=False,
        enable_asserts=False,
        num_devices=NCORES,
    )
    text = nc.dram_tensor("text", [BL, L, D], F32, kind="ExternalInput").ap()
    side = nc.dram_tensor("side", [128, BL * NLC * NS], BF16, kind="ExternalInput").ap()
    wa = nc.dram_tensor("wa", [128, NDC * H2], BF16, kind="ExternalInput").ap()
    wb = nc.dram_tensor("wb", [128, NDC * H2], F32, kind="ExternalInput").ap()
    bias2 = nc.dram_tensor("bias2", [1, H2], F32, kind="ExternalInput").ap()
    cntinv = nc.dram_tensor("cntinv", [BL, 1], F32, kind="ExternalInput").ap()
    ident8 = nc.dram_tensor("ident8", [BL, BL], F32, kind="ExternalInput").ap()
    identb = nc.dram_tensor("identb", [128, 128], BF16, kind="ExternalInput").ap()
    m12 = nc.dram_tensor("m12", [H2, 2], F32, kind="ExternalInput").ap()
    ones_row = nc.dram_tensor("ones_row", [1, BL], F32, kind="ExternalInput").ap()

    g_out = nc.dram_tensor("g_out", [H2, D], F32, kind="ExternalOutput").ap()
    u_out = nc.dram_tensor("u_out", [BL, H2], F32, kind="ExternalOutput").ap()
    ln_out = nc.dram_tensor("ln_out", [2, BL], F32, kind="ExternalOutput").ap()

    with tile.TileContext(nc) as tc:
        with ExitStack() as octx:
            const = octx.enter_context(tc.tile_pool(name="const", bufs=1))
            ep = octx.enter_context(tc.tile_pool(name="ep", bufs=BL))
            psS = octx.enter_context(tc.tile_pool(name="psS", bufs=1, space="PSUM"))

            ps_side = psS.tile([NS, D], F32)
            e_tiles = []

            with ExitStack() as p1:
                natp = p1.enter_context(tc.tile_pool(name="nat", bufs=4))
                ttp = p1.enter_context(tc.tile_pool(name="tt", bufs=4))
                psH = p1.enter_context(tc.tile_pool(name="psH", bufs=2, space="PSUM"))
                psT = p1.enter_context(tc.tile_pool(name="psT", bufs=2, space="PSUM"))
                ident_b = const.tile([128, 128], BF16)
                nc.sync.dma_start(out=ident_b, in_=identb)

                # first text tile in flight before the aux loads
                natb0 = natp.tile([128, NLC * D], BF16, tag="nat")
                nc.gpsimd.dma_start(
                    out=natb0.rearrange("p (lc d) -> p lc d", lc=NLC),
                    in_=text[0].rearrange("(lc p) d -> p lc d", p=128),
                )

                side_s = const.tile([128, BL * NLC * NS], BF16)
                nc.sync.dma_start(out=side_s, in_=side)
                wa_s = const.tile([128, NDC * H2], BF16)
                nc.sync.dma_start(out=wa_s, in_=wa)
                wb_s = const.tile([128, NDC * H2], F32)
                nc.sync.dma_start(out=wb_s, in_=wb)
                bias2_s = const.tile([1, H2], F32)
                nc.sync.dma_start(out=bias2_s, in_=bias2)
                cntinv_s = const.tile([BL, 1], F32)
                nc.sync.dma_start(out=cntinv_s, in_=cntinv)
                ident8_s = const.tile([BL, BL], F32)
                nc.sync.dma_start(out=ident8_s, in_=ident8)
                m12_s = const.tile([H2, 2], F32)
                nc.sync.dma_start(out=m12_s, in_=m12)
                ones_row_s = const.tile([1, BL], F32)
                nc.sync.dma_start(out=ones_row_s, in_=ones_row)

                for b in range(BL):
                    if b == 0:
                        natb = natb0
                    else:
                        natb = natp.tile([128, NLC * D], BF16, tag="nat")
                        nc.gpsimd.dma_start(
                            out=natb.rearrange("p (lc d) -> p lc d", lc=NLC),
                            in_=text[b].rearrange("(lc p) d -> p lc d", p=128),
                        )
                    # transpose: slot k=lc*4+dc <- cols [k*128,(k+1)*128)
                    ttbk = ttp.tile([128, NLC * NDC, 128], BF16, tag="tt")
                    if b % 2 == 0:
                        nc.sync.dma_start(out=ttbk, in_=natb, transpose=True)
                    else:
                        tk4 = ttbk.rearrange("p (lc dc) l -> p lc dc l", dc=NDC)
                        for dc in range(NDC):
                            pt = psT.tile([128, L], BF16, tag="pt")
                            for lc in range(NLC):
                                nc.tensor.transpose(
                                    pt[:, lc * 128 : (lc + 1) * 128],
                                    natb.rearrange("p (lc d) -> p lc d", lc=NLC)[
                                        :, lc, dc * 128 : (dc + 1) * 128
                                    ],
                                    ident_b,
                                )
                            nc.any.tensor_copy(
                                tk4[:, :, dc, :],
                                pt.rearrange("p (lc l) -> p lc l", lc=NLC),
                            )
                    # side-pass: accumulate [108, 512] over all (b, lc)
                    for lc in range(NLC):
                        t = b * NLC + lc
                        nc.tensor.matmul(
                            ps_side,
                            lhsT=side_s[:, t * NS : (t + 1) * NS],
                            rhs=natb[:, lc * D : (lc + 1) * D],
                            start=(t == 0),
                            stop=(t == BL * NLC - 1),
                        )
                    # heads: T^T[100c, 512l] = sum_dc W''_dc.T @ textT_dc
                    ph = psH.tile([H2, L], F32, tag="ph")
                    tt4 = ttbk.rearrange("p (lc dc) l -> p lc dc l", dc=NDC)
                    for dc in range(NDC):
                        nc.tensor.matmul(
                            ph,
                            lhsT=wa_s[:, dc * H2 : (dc + 1) * H2],
                            rhs=tt4[:, :, dc, :],
                            start=(dc == 0),
                            stop=(dc == NDC - 1),
                        )
                    e_b = ep.tile([H2, L], F32R, tag="E")
                    nc.scalar.activation(e_b, ph, mybir.ActivationFunctionType.Exp)
                    e_tiles.append(e_b)

            with ExitStack() as p2:
                smallp = p2.enter_context(tc.tile_pool(name="small", bufs=2))
                psU = p2.enter_context(tc.tile_pool(name="psU", bufs=2, space="PSUM"))
                psE = p2.enter_context(tc.tile_pool(name="psE", bufs=3, space="PSUM"))

                # sbj_vec = sbj_sum / cnt  [8, 512]
                sbj = const.tile([BL, D], F32)
                nc.vector.tensor_scalar_mul(sbj, ps_side[0:BL, :], cntinv_s)
                # transpose sbj -> [128d x 4, 8]
                pstT = psU.tile([128, NDC * BL], F32, tag="u")
                for dc in range(NDC):
                    nc.tensor.transpose(
                        pstT[:, dc * BL : (dc + 1) * BL],
                        sbj[:, dc * 128 : (dc + 1) * 128],
                        ident8_s,
                    )
                sbjT_s = const.tile([128, NDC * BL], F32)
                nc.any.tensor_copy(sbjT_s, pstT)
                # u = sbj_vec @ [W1b|W2b] + bias   [8, 100]
                pu = psU.tile([BL, H2], F32, tag="u")
                for dc in range(NDC):
                    nc.tensor.matmul(
                        pu,
                        lhsT=sbjT_s[:, dc * BL : (dc + 1) * BL],
                        rhs=wb_s[:, dc * H2 : (dc + 1) * H2],
                        start=(dc == 0),
                        stop=False,
                    )
                nc.tensor.matmul(
                    pu, lhsT=ones_row_s, rhs=bias2_s, start=False, stop=True
                )
                uS = const.tile([BL, H2], F32)
                nc.any.tensor_copy(uS, pu)
                nc.sync.dma_start(out=u_out, in_=uS)
                # w = exp(u^T)  [100, 8]
                puT = psU.tile([H2, BL], F32, tag="u")
                nc.tensor.transpose(puT, uS, ident8_s)
                w_s = const.tile([H2, BL], F32)
                nc.scalar.activation(w_s, puT, mybir.ActivationFunctionType.Exp)

                lnacc = const.tile([2, BL], F32)
                for b in range(BL):
                    wsel = smallp.tile([H2, 2], F32R, tag="wsel")
                    nc.vector.tensor_scalar_mul(wsel, m12_s, w_s[:, b : b + 1])
                    pe_ = psE.tile([2, L], F32, tag="e")
                    nc.tensor.matmul(
                        pe_, lhsT=wsel, rhs=e_tiles[b], start=True, stop=True
                    )
                    lnscr = smallp.tile([2, L], F32, tag="lnscr")
                    nc.scalar.activation(
                        lnscr,
                        pe_,
                        mybir.ActivationFunctionType.Ln,
                        accum_out=lnacc[:, b : b + 1],
                    )
                nc.sync.dma_start(out=ln_out, in_=lnacc)

                gc = const.tile([NS, D], F32)
                nc.any.tensor_copy(gc, ps_side)
                nc.sync.dma_start(out=g_out, in_=gc[BL:NS, :])

    nc.compile()
    return nc


def _get_program():
    if "nc" not in _CACHE:
        _CACHE["nc"] = _build_program()
    return _CACHE["nc"]


def _host_prep(text_vec, sbj_bound, obj_start, obj_end, W_start, b_start, W_end, b_end):
    """Build per-core input maps."""
    text_vec = np.ascontiguousarray(np.asarray(text_vec, dtype=np.float32))
    sbj = np.asarray(sbj_bound).astype(np.int64)
    objs = np.asarray(obj_start).astype(np.int64)
    obje = np.asarray(obj_end).astype(np.int64)
    W_start = np.asarray(W_start, dtype=np.float32)
    W_end = np.asarray(W_end, dtype=np.float32)
    b_start = np.asarray(b_start, dtype=np.float32)
    b_end = np.asarray(b_end, dtype=np.float32)

    wa_cat = np.concatenate([W_start[:D], W_end[:D]], axis=1)  # [512, 100]
    wb_cat = np.concatenate([W_start[D:], W_end[D:]], axis=1)  # [512, 100]
    wa_h = np.ascontiguousarray(
        wa_cat.reshape(NDC, 128, H2).transpose(1, 0, 2).reshape(128, NDC * H2)
    ).astype(BF16NP)
    wb_h = np.ascontiguousarray(
        wb_cat.reshape(NDC, 128, H2).transpose(1, 0, 2).reshape(128, NDC * H2)
    )
    bias2 = np.concatenate([b_start, b_end])[None, :].astype(np.float32)
    ident8 = np.eye(BL, dtype=np.float32)
    identb = np.eye(128, dtype=BF16NP)
    m12 = np.zeros((H2, 2), dtype=np.float32)
    m12[:C, 0] = 1.0
    m12[C:, 1] = 1.0
    ones_row = np.ones((1, BL), dtype=np.float32)

    pos = np.arange(L)
    span_all = (
        (pos[None, :] >= sbj[:, 0:1]) & (pos[None, :] <= sbj[:, 1:2])
    ).astype(np.float32)  # [B, L]
    cnt_all = span_all.sum(axis=1)  # [B]

    in_maps = []
    for c in range(NCORES):
        gb = slice(c * BL, (c + 1) * BL)
        # side stationary [t = b*4+lc][p][j]
        side_t = np.zeros((BL * NLC, 128, NS), dtype=np.float32)
        for b in range(BL):
            g = c * BL + b
            for lc in range(NLC):
                rows = slice(lc * 128, (lc + 1) * 128)
                t = b * NLC + lc
                side_t[t, :, b] = span_all[g, rows]
                ls = objs[g, rows]
                le = obje[g, rows]
                side_t[t, np.arange(128), BL + ls] = 1.0
                side_t[t, np.arange(128), BL + C + le] = 1.0
        side_h = np.ascontiguousarray(
            side_t.transpose(1, 0, 2).reshape(128, BL * NLC * NS)
        ).astype(BF16NP)
        cntinv = (1.0 / cnt_all[gb]).astype(np.float32)[:, None]
        in_maps.append(
            {
                "text": text_vec[gb],
                "side": side_h,
                "wa": wa_h,
                "wb": wb_h,
                "bias2": bias2,
                "cntinv": cntinv,
                "ident8": ident8,
                "identb": identb,
                "m12": m12,
                "ones_row": ones_row,
            }
        )
    return in_maps


def kernel(
    text_vec,
    text_mask,
    sbj_bound,
    obj_start,
    obj_end,
    W_start,
    b_start,
    W_end,
    b_end,
):
    text_mask = np.asarray(text_mask)
    if not bool(text_mask.all()):
        # Spec guarantees all-ones mask; numpy fallback for generality.
        return _numpy_reference(
            text_vec, text_mask, sbj_bound, obj_start, obj_end,
            W_start, b_start, W_end, b_end,
        )

    nc = _get_program()
    in_maps = _host_prep(
        text_vec, sbj_bound, obj_start, obj_end, W_start, b_start, W_end, b_end
    )
    res = run_bass_kernel_spmd(nc, in_maps, core_ids=list(range(NCORES)))

    W_start = np.asarray(W_start, dtype=np.float32)
    W_end = np.asarray(W_end, dtype=np.float32)
    objs = np.asarray(obj_start).astype(np.int64)
    obje = np.asarray(obj_end).astype(np.int64)

    w1aT = W_start[:D].T.astype(np.float64)  # [50, 512]
    w2aT = W_end[:D].T.astype(np.float64)

    total = 0.0
    for c in range(NCORES):
        r = res.results[c]
        ln_sum = float(r["ln_out"].astype(np.float64).sum())
        g = r["g_out"].astype(np.float64)  # [100, 512]
        gather_t = float((g[:C] * w1aT).sum() + (g[C:] * w2aT).sum())
        u = r["u_out"].astype(np.float64)  # [8, 100]
        u_term = 0.0
        for b in range(BL):
            gbi = c * BL + b
            cnt1 = np.bincount(objs[gbi], minlength=C).astype(np.float64)
            cnt2 = np.bincount(obje[gbi], minlength=C).astype(np.float64)
            u_term += float((cnt1 * u[b, :C]).sum() + (cnt2 * u[b, C:]).sum())
        total += ln_sum - gather_t - u_term

    value_num = float(text_mask.sum())
    return np.array(total / value_num, dtype=np.float32)


def _numpy_reference(
    text_vec, text_mask, sbj_bound, obj_start, obj_end, W_start, b_start, W_end, b_end
):
    text_vec = np.asarray(text_vec, dtype=np.float32)
    maskf = np.asarray(text_mask).astype(np.float32)
    sbj = np.asarray(sbj_bound).astype(np.int64)
    objs = np.asarray(obj_start).astype(np.int64)
    obje = np.asarray(obj_end).astype(np.int64)
    W_start = np.asarray(W_start, dtype=np.float32)
    W_end = np.asarray(W_end, dtype=np.float32)
    b_start = np.asarray(b_start, dtype=np.float32)
    b_end = np.asarray(b_end, dtype=np.float32)

    pos = np.arange(L)
    span = (
        (pos[None, :] >= sbj[:, 0:1]) & (pos[None, :] <= sbj[:, 1:2])
    ).astype(np.float32)
    count = span.sum(axis=1, keepdims=True)
    sbj_vec = np.einsum("bl,bld->bd", span, text_vec) / count

    def head(W, bv):
        return (
            np.einsum("bld,dc->blc", text_vec, W[:D]) + (sbj_vec @ W[D:])[:, None, :] + bv
        )

    def masked_ce(logits, labels, maskf, vn):
        m = logits.max(axis=-1, keepdims=True)
        logp = logits - m - np.log(np.exp(logits - m).sum(axis=-1, keepdims=True))
        nll = -np.take_along_axis(logp, labels[..., None], axis=-1)[..., 0]
        return (nll * maskf).sum() / vn

    vn = maskf.sum()
    o1 = head(W_start, b_start)
    o2 = head(W_end, b_end)
    return np.array(
        masked_ce(o1, objs, maskf, vn) + masked_ce(o2, obje, maskf, vn),
        dtype=np.float32,
    )
